# revision 5
# baseline (speedup 1.0000x reference)
# Bidirectional LSTM encoder (Keras-style, mask_zero) on 8 Trainium2 NeuronCores.
#
# Problem: x[64,512] int32 tokens -> embedding[32000,256] -> BiLSTM(512 units)
#   returns (encoder_output[64,512,1024], state_h[64,1024], state_c[64,1024])
#
# Strategy (SPMD, one program, per-core inputs differ):
#   - cores 0-3: forward direction, batch shards of 16
#   - cores 4-7: backward direction (host pre-reverses time), same batch shards
#   - per core, phase 1: embedding gather (indirect DMA) -> PE transpose ->
#     Zx = emb @ W + b for all timesteps as one big matmul (bf16), staged to DRAM
#     in transposed layout [gate-unit on partitions, (t,b) on free]
#   - phase 2: sequential recurrence over T steps; per step the PE streams
#     U (64 [128,128] bf16 tiles, FWL) against h^T [128, 16]; gates evaluated
#     with tanh-only activations (sigmoid via 0.5*(1+tanh(x/2)) to avoid
#     ACT table swaps); masked state update via copy_predicated.
#
# Layouts (per core):
#   unit u in [0,512) lives at (sub s = u//128, partition p = u%128)
#   z free index = g*4*BL + s*BL + b,  gate g in {i=0, f=1, o=2, g_cell=3}
#   U/W columns host-permuted accordingly; U rows natural (k-chunk s = row//128)

import numpy as np
import ml_dtypes
from contextlib import ExitStack

import concourse.bass as bass
import concourse.mybir as mybir
import concourse.tile as tile
from concourse import bacc
from concourse.bass import ds, ts
from concourse.bass_utils import run_bass_kernel_spmd
from concourse.masks import make_identity
from concourse.ap import AP

BF16 = mybir.dt.bfloat16
F32 = mybir.dt.float32
I32 = mybir.dt.int32
NPBF = ml_dtypes.bfloat16

V, E, U = 32000, 256, 512
BT, T = 64, 512          # full batch, full seq len
NCORES = 8
BL = BT // 4             # 16 sequences per core (4 cores per direction)
NM = (4 * U) // 128      # 16 column chunks of the 4U gate dim
KU = U // 128            # 4 k-chunks for the recurrent matmul
KE = E // 128            # 2 k-chunks for the input matmul
ACT_F = mybir.ActivationFunctionType

# Keras gate order in weights is (i, f, g, o); our column-block order is
# (i, f, o, g_cell) so sigmoid-gates are contiguous.
GATE_BASE = [0, U, 3 * U, 2 * U]  # orig col base for our gate index 0..3


def _col_perm():
    perm = np.empty(4 * U, dtype=np.int64)
    for m in range(NM):
        g, s = divmod(m, 4)
        base = GATE_BASE[g] + s * 128
        perm[m * 128:(m + 1) * 128] = np.arange(base, base + 128)
    return perm


def pack_weights(W, Uk, b):
    """-> wt [128, KE*NM*128] bf16, ut [128, KU*NM*128] bf16, bias [128, NM] f32"""
    perm = _col_perm()
    Wp = np.asarray(W)[:, perm]
    Up = np.asarray(Uk)[:, perm]
    bp = np.asarray(b)[perm]
    wt = Wp.reshape(KE, 128, 4 * U).transpose(1, 0, 2).reshape(128, KE * 4 * U)
    ut = Up.reshape(KU, 128, 4 * U).transpose(1, 0, 2).reshape(128, KU * 4 * U)
    bias = bp.reshape(NM, 128).T.copy()
    return (np.ascontiguousarray(wt).astype(NPBF),
            np.ascontiguousarray(ut).astype(NPBF),
            np.ascontiguousarray(bias).astype(np.float32))


def build_kernel(tc, Tn=T, BLn=BL, Vn=V, unroll=8):
    nc = tc.nc
    SB = KU * BLn            # per-gate free width in z
    ZW = NM * BLn            # total z free width
    NTOK = Tn * BLn
    NG = NTOK // 128         # gather tiles
    NZ = NTOK // 512         # 512-wide column blocks for the Zx matmul
    TW = 512 // BLn          # timesteps covered per 512-wide block

    xidx = nc.dram_tensor("xidx", [NG, 128], I32, kind="ExternalInput")
    emb = nc.dram_tensor("emb", [Vn, E], F32, kind="ExternalInput")
    wt_d = nc.dram_tensor("wt", [128, KE * 4 * U], BF16, kind="ExternalInput")
    ut_d = nc.dram_tensor("ut", [128, KU * 4 * U], BF16, kind="ExternalInput")
    bias_d = nc.dram_tensor("bias", [128, NM], F32, kind="ExternalInput")
    maskx = nc.dram_tensor("maskx", [Tn, 128, KU * BLn], mybir.dt.uint8,
                           kind="ExternalInput")
    zx_d = nc.dram_tensor("zx", [NM, 128, Tn, BLn], BF16, kind="Internal")
    hs_d = nc.dram_tensor("hs", [Tn, KU, 128, BLn], F32, kind="ExternalOutput")
    hT_d = nc.dram_tensor("hT", [KU, 128, BLn], F32, kind="ExternalOutput")
    cT_d = nc.dram_tensor("cT", [KU, 128, BLn], F32, kind="ExternalOutput")

    with ExitStack() as ctx:
        cpool = ctx.enter_context(tc.tile_pool(name="consts", bufs=1))
        ident = cpool.tile([128, 128], F32)
        make_identity(nc, ident[:])
        wt_sb = cpool.tile([128, KE * 4 * U], BF16)
        nc.sync.dma_start(wt_sb[:], wt_d.ap())
        ut_sb = cpool.tile([128, KU * 4 * U], BF16)
        nc.sync.dma_start(ut_sb[:], ut_d.ap())
        bias_sb = cpool.tile([128, NM], F32)
        nc.sync.dma_start(bias_sb[:], bias_d.ap())

        # ---------------- phase 1: gather + transpose + Zx ----------------
        embT = [cpool.tile([128, NTOK], BF16, name=f"embT{e}", tag=f"embT{e}")
                for e in range(KE)]
        with tc.tile_pool(name="ph1", bufs=4) as gp, \
             tc.tile_pool(name="ph1ps", bufs=8, space="PSUM") as pp1:
            for g in range(NG):
                idxt = gp.tile([128, 1], I32, tag="idx")
                nc.sync.dma_start(idxt[:], xidx.ap()[g])
                gat = gp.tile([128, E], F32, tag="gat")
                nc.gpsimd.indirect_dma_start(
                    out=gat[:], out_offset=None, in_=emb.ap(),
                    in_offset=bass.IndirectOffsetOnAxis(ap=idxt[:, :1], axis=0))
                for e in range(KE):
                    tp = pp1.tile([128, 128], F32, tag="tp")
                    nc.tensor.transpose(out=tp[:], in_=gat[:, ts(e, 128)],
                                        identity=ident[:])
                    if (2 * g + e) % 2 == 0:
                        nc.scalar.copy(embT[e][:, ts(g, 128)], tp[:])
                    else:
                        nc.vector.tensor_copy(embT[e][:, ts(g, 128)], tp[:])

        with tc.tile_pool(name="zxe", bufs=4) as zp, \
             tc.tile_pool(name="zxps", bufs=4, space="PSUM") as pp2:
            for n in range(NZ):
                for m in range(NM):
                    px = pp2.tile([128, 512], F32, tag="px")
                    for k in range(KE):
                        nc.tensor.matmul(
                            px[:], wt_sb[:, (k * NM + m) * 128:(k * NM + m + 1) * 128],
                            embT[k][:, n * 512:(n + 1) * 512],
                            start=(k == 0), stop=(k == KE - 1))
                    ze = zp.tile([128, 512], BF16, tag="ze")
                    if m % 2 == 0:
                        nc.vector.tensor_scalar_add(ze[:], px[:], bias_sb[:, m:m + 1])
                    else:
                        nc.scalar.activation(ze[:], px[:], ACT_F.Identity,
                                             bias=bias_sb[:, m:m + 1])
                    nc.sync.dma_start(
                        zx_d.ap()[m, :, n * TW:(n + 1) * TW, :],
                        ze[:].rearrange("p (t b) -> p t b", t=TW))

        # ---------------- phase 2: the recurrence ----------------
        spool = ctx.enter_context(tc.tile_pool(name="state", bufs=1))
        h_sb = spool.tile([128, SB], F32)
        c_sb = spool.tile([128, SB], F32)
        hbf = spool.tile([128, SB], BF16)
        nc.vector.memset(h_sb[:], 0.0)
        nc.vector.memset(c_sb[:], 0.0)
        nc.vector.memset(hbf[:], 0.0)

        zx_r = zx_d.ap().rearrange("m p t b -> p m t b")
        hs_r = hs_d.ap().rearrange("t s p b -> t p s b")

        lp = ctx.enter_context(tc.tile_pool(name="loop", bufs=3))
        pzp = ctx.enter_context(tc.tile_pool(name="loopps", bufs=2, space="PSUM"))

        def step(t):
            zxt = lp.tile([128, ZW], BF16, tag="zxt")
            nc.sync.dma_start(zxt[:].rearrange("p (m b) -> p m b", m=NM),
                              zx_r[:, :, ds(t, 1), :])
            mt = lp.tile([128, SB], mybir.dt.uint8, tag="mt")
            nc.sync.dma_start(mt[:], maskx.ap()[ds(t, 1)])

            # Preload z = Zx into PSUM, then accumulate U @ h on top with
            # start=False matmuls (per-element accumulate, no zero-region
            # poisoning -> 16 interleaved column groups in one bank are fine).
            psz = pzp.tile([128, ZW], F32, tag="psz")
            nc.vector.tensor_copy(psz[:], zxt[:])
            for k in range(KU):
                for m in range(NM):
                    nc.tensor.matmul(
                        psz[:, m * BLn:(m + 1) * BLn],
                        ut_sb[:, (k * NM + m) * 128:(k * NM + m + 1) * 128],
                        hbf[:, k * BLn:(k + 1) * BLn],
                        start=False, stop=False, skip_group_check=True)

            gt = lp.tile([128, ZW], F32, tag="gt")
            # tanh-only: gates i,f,o get tanh(z/2); cell gate g gets tanh(z)
            nc.scalar.activation(gt[:, 0:3 * SB], psz[:, 0:3 * SB], ACT_F.Tanh,
                                 scale=0.5)
            nc.scalar.activation(gt[:, 3 * SB:4 * SB], psz[:, 3 * SB:4 * SB],
                                 ACT_F.Tanh)
            # sigmoid reconstruction: s = 0.5*(1+t)
            sg = lp.tile([128, 3 * SB], F32, tag="sg")
            nc.vector.tensor_scalar(sg[:], gt[:, 0:3 * SB], 1.0, 0.5,
                                    op0=mybir.AluOpType.add,
                                    op1=mybir.AluOpType.mult)
            ig = lp.tile([128, SB], F32, tag="ig")
            nc.vector.tensor_tensor(out=ig[:], in0=sg[:, 0:SB],
                                    in1=gt[:, 3 * SB:4 * SB],
                                    op=mybir.AluOpType.mult)
            fc = lp.tile([128, SB], F32, tag="fc")
            nc.vector.tensor_tensor(out=fc[:], in0=sg[:, SB:2 * SB], in1=c_sb[:],
                                    op=mybir.AluOpType.mult)
            cn = lp.tile([128, SB], F32, tag="cn")
            nc.vector.tensor_add(cn[:], ig[:], fc[:])

            nc.vector.copy_predicated(c_sb[:], mt[:], cn[:])
            th = lp.tile([128, SB], F32, tag="th")
            nc.scalar.activation(th[:], c_sb[:], ACT_F.Tanh)
            hn = lp.tile([128, SB], F32, tag="hn")
            nc.vector.tensor_tensor(out=hn[:], in0=sg[:, 2 * SB:3 * SB],
                                    in1=th[:], op=mybir.AluOpType.mult)
            nc.vector.copy_predicated(h_sb[:], mt[:], hn[:])
            nc.vector.tensor_copy(hbf[:], h_sb[:])
            nc.sync.dma_start(hs_r[ds(t, 1)],
                              h_sb[:].rearrange("p (s b) -> p s b", s=KU))

        if unroll <= 1:
            with tc.For_i(0, Tn, 1) as iv:
                step(iv)
        else:
            assert Tn % unroll == 0
            with tc.For_i(0, Tn, unroll,
                          hint_engines=(mybir.EngineType.PE,)) as iv:
                for u in range(unroll):
                    step(iv + u)

        nc.sync.dma_start(hT_d.ap().rearrange("s p b -> p s b"),
                          h_sb[:].rearrange("p (s b) -> p s b", s=KU))
        nc.sync.dma_start(cT_d.ap().rearrange("s p b -> p s b"),
                          c_sb[:].rearrange("p (s b) -> p s b", s=KU))


_CACHE = {}


def get_compiled(Tn=T, BLn=BL, Vn=V, unroll=8):
    key = (Tn, BLn, Vn, unroll)
    if key not in _CACHE:
        nc = bacc.Bacc("TRN2", target_bir_lowering=False, debug=False,
                       enable_asserts=False, num_devices=NCORES)
        with tile.TileContext(nc) as tc:
            build_kernel(tc, Tn=Tn, BLn=BLn, Vn=Vn, unroll=unroll)
        nc.compile()
        _CACHE[key] = nc
    return _CACHE[key]


def make_in_maps(x, emb_table, Wf, Uf, bf, Wb, Ub, bb, Tn=T, BLn=BL):
    """Build the 8 per-core input dicts."""
    NG = (Tn * BLn) // 128
    emb_f32 = np.ascontiguousarray(emb_table, dtype=np.float32)
    packs = [pack_weights(Wf, Uf, bf), pack_weights(Wb, Ub, bb)]
    ncore_per_dir = 4
    in_maps = []
    for c in range(NCORES):
        d = c // ncore_per_dir
        bsl = slice((c % ncore_per_dir) * BLn, (c % ncore_per_dir + 1) * BLn)
        xc = np.asarray(x)[bsl]
        if d == 1:
            xc = xc[:, ::-1]
        xt = np.ascontiguousarray(xc.T.astype(np.int32))       # [T, BL]
        xidx = xt.reshape(NG, 128)
        mask = (xt != 0).astype(np.uint8)                       # [T, BL]
        maskx = np.ascontiguousarray(np.broadcast_to(
            mask[:, None, None, :], (Tn, 128, KU, BLn))).reshape(Tn, 128, KU * BLn)
        wt, ut, bias = packs[d]
        in_maps.append({"xidx": xidx, "emb": emb_f32, "wt": wt, "ut": ut,
                        "bias": bias, "maskx": maskx})
    return in_maps


def assemble(results, Tn=T, BLn=BL):
    enc = np.empty((BT, Tn, 2 * U), dtype=np.float32)
    state_h = np.empty((BT, 2 * U), dtype=np.float32)
    state_c = np.empty((BT, 2 * U), dtype=np.float32)
    ncore_per_dir = 4
    for c in range(NCORES):
        d = c // ncore_per_dir
        bsl = slice((c % ncore_per_dir) * BLn, (c % ncore_per_dir + 1) * BLn)
        hs = results[c]["hs"]          # [T, KU, 128, BL]
        arr = hs.transpose(3, 0, 1, 2).reshape(BLn, Tn, U)
        if d == 1:
            arr = arr[:, ::-1]
        enc[bsl, :, d * U:(d + 1) * U] = arr
        state_h[bsl, d * U:(d + 1) * U] = \
            results[c]["hT"].transpose(2, 0, 1).reshape(BLn, U)
        state_c[bsl, d * U:(d + 1) * U] = \
            results[c]["cT"].transpose(2, 0, 1).reshape(BLn, U)
    return enc, state_h, state_c


def kernel(x, emb_table, Wf, Uf, bf, Wb, Ub, bb):
    nc = get_compiled()
    in_maps = make_in_maps(x, emb_table, Wf, Uf, bf, Wb, Ub, bb)
    res = run_bass_kernel_spmd(nc, in_maps, core_ids=list(range(NCORES)))
    return assemble(res.results)
